# revision 4
# baseline (speedup 1.0000x reference)
"""Trainium2 Bass kernel for nn_MultLayerAdaptiveSimple.

Computes out = X * W[idx, 0] + Y * W[idx, 1] where idx = reward[..., 0]
(values in {0, 1}), X/Y: [4, 4096, 2048] f32, W: [2, 2] f32.

Sharding: pure data-parallel over the flattened (B*S) row axis across 8
NeuronCores; the 2x2 table is replicated. Each core processes 2048 rows
of 2048 f32 elements (16 MB per tensor per core).

Device work per core:
  - per-row blend weights a = W[idx,0], b = W[idx,1] computed exactly on
    DVE via a = (1-idx)*W00 + idx*W10 (idx in {0,1} so each product is
    exact), using per-partition scalar operands.
  - per 128-row chunk: ACT does y *= b (activation Copy with per-partition
    scale), DVE does x = (x * a) + y in one fused scalar_tensor_tensor.
  - HWDGE (nc.sync) moves 4 MB tiles HBM<->SBUF.
"""

import numpy as np

import concourse.bacc as bacc
import concourse.bass as bass
import concourse.mybir as mybir
from concourse.bass_utils import run_bass_kernel_spmd
from concourse.tile import TileContext

B, S, D = 4, 4096, 2048
N_CORES = 8
ROWS = B * S                      # 16384
ROWS_PER_CORE = ROWS // N_CORES   # 2048
P = 128                           # SBUF partitions
GROUPS = ROWS_PER_CORE // P       # 16 row-groups of 128 rows per core
CH = 4                            # row-groups per DMA tile (4 MB tiles)
N_TILES = GROUPS // CH

F32 = mybir.dt.float32
MULT = mybir.AluOpType.mult
ADD = mybir.AluOpType.add


def _build_bass() -> bass.Bass:
    nc = bacc.Bacc(trn_type="TRN2", debug=False)

    x = nc.dram_tensor("x", [ROWS_PER_CORE, D], F32, kind="ExternalInput").ap()
    y = nc.dram_tensor("y", [ROWS_PER_CORE, D], F32, kind="ExternalInput").ap()
    idx = nc.dram_tensor("idx", [P, GROUPS], F32, kind="ExternalInput").ap()
    w = nc.dram_tensor("w", [P, 4], F32, kind="ExternalInput").ap()
    out = nc.dram_tensor("out", [ROWS_PER_CORE, D], F32, kind="ExternalOutput").ap()

    # Tile t covers rows [t*CH*P, (t+1)*CH*P): partition p of chunk c holds
    # row (t*CH + c)*P + p. Row-group index g = t*CH + c matches idx[:, g].
    xv = x.rearrange("(t c p) d -> t p c d", c=CH, p=P)
    yv = y.rearrange("(t c p) d -> t p c d", c=CH, p=P)
    ov = out.rearrange("(t c p) d -> t p c d", c=CH, p=P)

    with TileContext(nc) as tc:
        with (
            tc.tile_pool(name="small", bufs=1) as small,
            tc.tile_pool(name="xp", bufs=2) as xp,
            tc.tile_pool(name="yp", bufs=2) as yp,
        ):
            idx_t = small.tile([P, GROUPS], F32)
            w_t = small.tile([P, 4], F32)
            nc.sync.dma_start(out=idx_t[:], in_=idx)
            nc.sync.dma_start(out=w_t[:], in_=w)

            # nidx = 1 - idx (exact for idx in {0,1})
            nidx_t = small.tile([P, GROUPS], F32)
            nc.vector.tensor_scalar(nidx_t[:], idx_t[:], -1.0, 1.0, MULT, ADD)

            # a = nidx*W00 + idx*W10 ; b = nidx*W01 + idx*W11   (all exact)
            ta = small.tile([P, GROUPS], F32)
            tb = small.tile([P, GROUPS], F32)
            a_t = small.tile([P, GROUPS], F32)
            b_t = small.tile([P, GROUPS], F32)
            nc.vector.tensor_scalar(ta[:], idx_t[:], w_t[:, 2:3], None, MULT)
            nc.vector.scalar_tensor_tensor(a_t[:], nidx_t[:], w_t[:, 0:1], ta[:], MULT, ADD)
            nc.vector.tensor_scalar(tb[:], idx_t[:], w_t[:, 3:4], None, MULT)
            nc.vector.scalar_tensor_tensor(b_t[:], nidx_t[:], w_t[:, 1:2], tb[:], MULT, ADD)

            for t in range(N_TILES):
                xt = xp.tile([P, CH * D], F32)
                yt = yp.tile([P, CH * D], F32)
                xt3 = xt[:].rearrange("p (c d) -> p c d", c=CH)
                yt3 = yt[:].rearrange("p (c d) -> p c d", c=CH)
                nc.sync.dma_start(out=xt3, in_=xv[t])
                nc.sync.dma_start(out=yt3, in_=yv[t])
                for c in range(CH):
                    g = t * CH + c
                    xs = xt[:, c * D : (c + 1) * D]
                    ys = yt[:, c * D : (c + 1) * D]
                    # y *= b   (ACT engine, per-partition scale)
                    nc.scalar.activation(
                        ys, ys, mybir.ActivationFunctionType.Copy,
                        scale=b_t[:, g : g + 1],
                    )
                    # x = x*a + y   (DVE, fused)
                    nc.vector.scalar_tensor_tensor(
                        xs, xs, a_t[:, g : g + 1], ys, MULT, ADD
                    )
                nc.sync.dma_start(out=ov[t], in_=xt3)

    nc.compile()
    return nc


def _shard_inputs(X, Y, reward, W):
    Xf = np.ascontiguousarray(np.asarray(X, dtype=np.float32).reshape(ROWS, D))
    Yf = np.ascontiguousarray(np.asarray(Y, dtype=np.float32).reshape(ROWS, D))
    idx_all = np.asarray(reward).reshape(ROWS).astype(np.float32)
    w_rep = np.ascontiguousarray(
        np.tile(np.asarray(W, dtype=np.float32).reshape(1, 4), (P, 1))
    )
    in_maps = []
    for k in range(N_CORES):
        sl = slice(k * ROWS_PER_CORE, (k + 1) * ROWS_PER_CORE)
        # idx_core[p, g] = idx of row g*P + p of this core's shard
        idx_core = np.ascontiguousarray(idx_all[sl].reshape(GROUPS, P).T)
        in_maps.append(
            {
                "x": np.ascontiguousarray(Xf[sl]),
                "y": np.ascontiguousarray(Yf[sl]),
                "idx": idx_core,
                "w": w_rep,
            }
        )
    return in_maps


def run(X, Y, reward, W, trace=False, tmpdir=None):
    """Build, run on 8 cores; returns (full_output, BassKernelResults)."""
    in_maps = _shard_inputs(X, Y, reward, W)
    nc = _build_bass()
    res = run_bass_kernel_spmd(
        nc, in_maps, core_ids=list(range(N_CORES)), trace=trace, tmpdir=tmpdir
    )
    shards = [res.results[k]["out"] for k in range(N_CORES)]
    full = np.concatenate(shards, axis=0).reshape(B, S, D)
    return full, res


def kernel(X, Y, reward, W):
    full, _ = run(X, Y, reward, W)
    return full


# revision 9
# speedup vs baseline: 1.1217x; 1.1217x over previous
"""Trainium2 Bass kernel for nn_MultLayerAdaptiveSimple.

Computes out = X * W[idx, 0] + Y * W[idx, 1] where idx = reward[..., 0]
(values in {0, 1}), X/Y: [4, 4096, 2048] f32, W: [2, 2] f32.

Sharding: pure data-parallel over the flattened (B*S) row axis across 8
NeuronCores; the 2x2 table is replicated. Each core processes 2048 rows
of 2048 f32 elements (16 MB per tensor per core).

Device work per core:
  - per-row blend weights a = W[idx,0], b = W[idx,1] computed exactly on
    DVE via a = (1-idx)*W00 + idx*W10 (idx in {0,1} so each product is
    exact), using per-partition scalar operands.
  - per 128-row chunk: ACT does y *= b (activation Copy with per-partition
    scale), DVE does x = (x * a) + y in one fused scalar_tensor_tensor.
  - HWDGE (nc.sync) moves 4 MB tiles HBM<->SBUF.
"""

import numpy as np

import concourse.bacc as bacc
import concourse.bass as bass
import concourse.mybir as mybir
from concourse.bass_utils import run_bass_kernel_spmd
from concourse.tile import TileContext

B, S, D = 4, 4096, 2048
N_CORES = 8
ROWS = B * S                      # 16384
ROWS_PER_CORE = ROWS // N_CORES   # 2048
P = 128                           # SBUF partitions
GROUPS = ROWS_PER_CORE // P       # 16 row-groups of 128 rows per core
CH = 2                            # row-groups per DMA tile (2 MB tiles)
N_TILES = GROUPS // CH

F32 = mybir.dt.float32
MULT = mybir.AluOpType.mult
ADD = mybir.AluOpType.add


def _build_bass() -> bass.Bass:
    nc = bacc.Bacc(trn_type="TRN2", debug=False, enable_partition_id=False)

    x = nc.dram_tensor("x", [ROWS_PER_CORE, D], F32, kind="ExternalInput").ap()
    y = nc.dram_tensor("y", [ROWS_PER_CORE, D], F32, kind="ExternalInput").ap()
    idx = nc.dram_tensor("idx", [P, GROUPS], F32, kind="ExternalInput").ap()
    w = nc.dram_tensor("w", [P, 4], F32, kind="ExternalInput").ap()
    out = nc.dram_tensor("out", [ROWS_PER_CORE, D], F32, kind="ExternalOutput").ap()

    # Tile t covers rows [t*CH*P, (t+1)*CH*P): partition p of chunk c holds
    # row (t*CH + c)*P + p. Row-group index g = t*CH + c matches idx[:, g].
    xv = x.rearrange("(t c p) d -> t p c d", c=CH, p=P)
    yv = y.rearrange("(t c p) d -> t p c d", c=CH, p=P)
    ov = out.rearrange("(t c p) d -> t p c d", c=CH, p=P)

    with TileContext(nc) as tc:
        with (
            tc.tile_pool(name="small", bufs=1) as small,
            tc.tile_pool(name="xp", bufs=4) as xp,
            tc.tile_pool(name="yp", bufs=4) as yp,
        ):
            idx_t = small.tile([P, GROUPS], F32)
            w_t = small.tile([P, 4], F32)
            nc.sync.dma_start(out=idx_t[:], in_=idx)
            nc.sync.dma_start(out=w_t[:], in_=w)

            # nidx = 1 - idx (exact for idx in {0,1})
            nidx_t = small.tile([P, GROUPS], F32)
            nc.vector.tensor_scalar(nidx_t[:], idx_t[:], -1.0, 1.0, MULT, ADD)

            # a = nidx*W00 + idx*W10 ; b = nidx*W01 + idx*W11   (all exact)
            ta = small.tile([P, GROUPS], F32)
            tb = small.tile([P, GROUPS], F32)
            a_t = small.tile([P, GROUPS], F32)
            b_t = small.tile([P, GROUPS], F32)
            nc.vector.tensor_scalar(ta[:], idx_t[:], w_t[:, 2:3], None, MULT)
            nc.vector.scalar_tensor_tensor(a_t[:], nidx_t[:], w_t[:, 0:1], ta[:], MULT, ADD)
            nc.vector.tensor_scalar(tb[:], idx_t[:], w_t[:, 3:4], None, MULT)
            nc.vector.scalar_tensor_tensor(b_t[:], nidx_t[:], w_t[:, 1:2], tb[:], MULT, ADD)

            for t in range(N_TILES):
                xt = xp.tile([P, CH * D], F32)
                yt = yp.tile([P, CH * D], F32)
                xt3 = xt[:].rearrange("p (c d) -> p c d", c=CH)
                yt3 = yt[:].rearrange("p (c d) -> p c d", c=CH)
                # x loads on the SP HWDGE ring, y loads on the ACT HWDGE
                # ring, stores on the SWDGE (gpsimd) queue: three DMA
                # streams that overlap instead of serializing in one FIFO.
                nc.sync.dma_start(out=xt3, in_=xv[t])
                nc.scalar.dma_start(out=yt3, in_=yv[t])
                for c in range(CH):
                    g = t * CH + c
                    xs = xt[:, c * D : (c + 1) * D]
                    ys = yt[:, c * D : (c + 1) * D]
                    # y *= b   (ACT engine, per-partition scale)
                    nc.scalar.activation(
                        ys, ys, mybir.ActivationFunctionType.Copy,
                        scale=b_t[:, g : g + 1],
                    )
                    # x = x*a + y   (DVE, fused)
                    nc.vector.scalar_tensor_tensor(
                        xs, xs, a_t[:, g : g + 1], ys, MULT, ADD
                    )
                nc.gpsimd.dma_start(out=ov[t], in_=xt3)

    nc.compile()
    return nc


def _shard_inputs(X, Y, reward, W):
    Xf = np.ascontiguousarray(np.asarray(X, dtype=np.float32).reshape(ROWS, D))
    Yf = np.ascontiguousarray(np.asarray(Y, dtype=np.float32).reshape(ROWS, D))
    idx_all = np.asarray(reward).reshape(ROWS).astype(np.float32)
    w_rep = np.ascontiguousarray(
        np.tile(np.asarray(W, dtype=np.float32).reshape(1, 4), (P, 1))
    )
    in_maps = []
    for k in range(N_CORES):
        sl = slice(k * ROWS_PER_CORE, (k + 1) * ROWS_PER_CORE)
        # idx_core[p, g] = idx of row g*P + p of this core's shard
        idx_core = np.ascontiguousarray(idx_all[sl].reshape(GROUPS, P).T)
        in_maps.append(
            {
                "x": np.ascontiguousarray(Xf[sl]),
                "y": np.ascontiguousarray(Yf[sl]),
                "idx": idx_core,
                "w": w_rep,
            }
        )
    return in_maps


def run(X, Y, reward, W, trace=False, tmpdir=None):
    """Build, run on 8 cores; returns (full_output, BassKernelResults)."""
    in_maps = _shard_inputs(X, Y, reward, W)
    nc = _build_bass()
    res = run_bass_kernel_spmd(
        nc, in_maps, core_ids=list(range(N_CORES)), trace=trace, tmpdir=tmpdir
    )
    shards = [res.results[k]["out"] for k in range(N_CORES)]
    full = np.concatenate(shards, axis=0).reshape(B, S, D)
    return full, res


def kernel(X, Y, reward, W):
    full, _ = run(X, Y, reward, W)
    return full


# revision 11
# speedup vs baseline: 1.1313x; 1.0086x over previous
"""Trainium2 Bass kernel for nn_MultLayerAdaptiveSimple.

Computes out = X * W[idx, 0] + Y * W[idx, 1] where idx = reward[..., 0]
(values in {0, 1}), X/Y: [4, 4096, 2048] f32, W: [2, 2] f32.

Sharding: pure data-parallel over the flattened (B*S) row axis across 8
NeuronCores; the 2x2 table is replicated. Each core processes 2048 rows
of 2048 f32 elements (16 MB per tensor per core).

Device work per core:
  - per-row blend weights a = W[idx,0], b = W[idx,1] computed exactly on
    DVE via a = (1-idx)*W00 + idx*W10 (idx in {0,1} so each product is
    exact), using per-partition scalar operands.
  - per 128-row chunk: ACT does y *= b (activation Copy with per-partition
    scale), DVE does x = (x * a) + y in one fused scalar_tensor_tensor.
  - HWDGE (nc.sync) moves 4 MB tiles HBM<->SBUF.
"""

import numpy as np

import concourse.bacc as bacc
import concourse.bass as bass
import concourse.mybir as mybir
from concourse.bass_utils import run_bass_kernel_spmd
from concourse.tile import TileContext

B, S, D = 4, 4096, 2048
N_CORES = 8
ROWS = B * S                      # 16384
ROWS_PER_CORE = ROWS // N_CORES   # 2048
P = 128                           # SBUF partitions
GROUPS = ROWS_PER_CORE // P       # 16 row-groups of 128 rows per core
CH = 2                            # row-groups per DMA tile (2 MB tiles)
N_TILES = GROUPS // CH

F32 = mybir.dt.float32
MULT = mybir.AluOpType.mult
ADD = mybir.AluOpType.add


def _build_bass() -> bass.Bass:
    nc = bacc.Bacc(trn_type="TRN2", debug=False, enable_partition_id=False)

    x = nc.dram_tensor("x", [ROWS_PER_CORE, D], F32, kind="ExternalInput").ap()
    y = nc.dram_tensor("y", [ROWS_PER_CORE, D], F32, kind="ExternalInput").ap()
    idx = nc.dram_tensor("idx", [P, GROUPS], F32, kind="ExternalInput").ap()
    w = nc.dram_tensor("w", [P, 4], F32, kind="ExternalInput").ap()
    out = nc.dram_tensor("out", [ROWS_PER_CORE, D], F32, kind="ExternalOutput").ap()

    # Tile t covers rows [t*CH*P, (t+1)*CH*P): partition p of chunk c holds
    # row (t*CH + c)*P + p. Row-group index g = t*CH + c matches idx[:, g].
    xv = x.rearrange("(t c p) d -> t p c d", c=CH, p=P)
    yv = y.rearrange("(t c p) d -> t p c d", c=CH, p=P)
    ov = out.rearrange("(g p) d -> g p d", p=P)  # per-chunk (1 MB) stores

    with TileContext(nc) as tc:
        with (
            tc.tile_pool(name="small", bufs=1) as small,
            tc.tile_pool(name="xp", bufs=4) as xp,
            tc.tile_pool(name="yp", bufs=4) as yp,
        ):
            idx_t = small.tile([P, GROUPS], F32)
            w_t = small.tile([P, 4], F32)
            nc.sync.dma_start(out=idx_t[:], in_=idx)
            nc.sync.dma_start(out=w_t[:], in_=w)

            # nidx = 1 - idx (exact for idx in {0,1})
            nidx_t = small.tile([P, GROUPS], F32)
            nc.vector.tensor_scalar(nidx_t[:], idx_t[:], -1.0, 1.0, MULT, ADD)

            # a = nidx*W00 + idx*W10 ; b = nidx*W01 + idx*W11   (all exact)
            ta = small.tile([P, GROUPS], F32)
            tb = small.tile([P, GROUPS], F32)
            a_t = small.tile([P, GROUPS], F32)
            b_t = small.tile([P, GROUPS], F32)
            nc.vector.tensor_scalar(ta[:], idx_t[:], w_t[:, 2:3], None, MULT)
            nc.vector.scalar_tensor_tensor(a_t[:], nidx_t[:], w_t[:, 0:1], ta[:], MULT, ADD)
            nc.vector.tensor_scalar(tb[:], idx_t[:], w_t[:, 3:4], None, MULT)
            nc.vector.scalar_tensor_tensor(b_t[:], nidx_t[:], w_t[:, 1:2], tb[:], MULT, ADD)

            for t in range(N_TILES):
                xt = xp.tile([P, CH * D], F32)
                yt = yp.tile([P, CH * D], F32)
                xt3 = xt[:].rearrange("p (c d) -> p c d", c=CH)
                yt3 = yt[:].rearrange("p (c d) -> p c d", c=CH)
                # x loads on the SP HWDGE ring, y loads on the ACT HWDGE
                # ring, stores on the SWDGE (gpsimd) queue: three DMA
                # streams that overlap instead of serializing in one FIFO.
                nc.sync.dma_start(out=xt3, in_=xv[t])
                nc.scalar.dma_start(out=yt3, in_=yv[t])
                for c in range(CH):
                    g = t * CH + c
                    xs = xt[:, c * D : (c + 1) * D]
                    ys = yt[:, c * D : (c + 1) * D]
                    # y *= b   (ACT engine, per-partition scale)
                    nc.scalar.activation(
                        ys, ys, mybir.ActivationFunctionType.Copy,
                        scale=b_t[:, g : g + 1],
                    )
                    # x = x*a + y   (DVE, fused)
                    nc.vector.scalar_tensor_tensor(
                        xs, xs, a_t[:, g : g + 1], ys, MULT, ADD
                    )
                    # store this chunk immediately (SWDGE queue)
                    nc.gpsimd.dma_start(out=ov[g], in_=xs)

    nc.compile()
    return nc


def _shard_inputs(X, Y, reward, W):
    Xf = np.ascontiguousarray(np.asarray(X, dtype=np.float32).reshape(ROWS, D))
    Yf = np.ascontiguousarray(np.asarray(Y, dtype=np.float32).reshape(ROWS, D))
    idx_all = np.asarray(reward).reshape(ROWS).astype(np.float32)
    w_rep = np.ascontiguousarray(
        np.tile(np.asarray(W, dtype=np.float32).reshape(1, 4), (P, 1))
    )
    in_maps = []
    for k in range(N_CORES):
        sl = slice(k * ROWS_PER_CORE, (k + 1) * ROWS_PER_CORE)
        # idx_core[p, g] = idx of row g*P + p of this core's shard
        idx_core = np.ascontiguousarray(idx_all[sl].reshape(GROUPS, P).T)
        in_maps.append(
            {
                "x": np.ascontiguousarray(Xf[sl]),
                "y": np.ascontiguousarray(Yf[sl]),
                "idx": idx_core,
                "w": w_rep,
            }
        )
    return in_maps


def run(X, Y, reward, W, trace=False, tmpdir=None):
    """Build, run on 8 cores; returns (full_output, BassKernelResults)."""
    in_maps = _shard_inputs(X, Y, reward, W)
    nc = _build_bass()
    res = run_bass_kernel_spmd(
        nc, in_maps, core_ids=list(range(N_CORES)), trace=trace, tmpdir=tmpdir
    )
    shards = [res.results[k]["out"] for k in range(N_CORES)]
    full = np.concatenate(shards, axis=0).reshape(B, S, D)
    return full, res


def kernel(X, Y, reward, W):
    full, _ = run(X, Y, reward, W)
    return full


# revision 13
# speedup vs baseline: 1.1537x; 1.0198x over previous
"""Trainium2 Bass kernel for nn_MultLayerAdaptiveSimple.

Computes out = X * W[idx, 0] + Y * W[idx, 1] where idx = reward[..., 0]
(values in {0, 1}), X/Y: [4, 4096, 2048] f32, W: [2, 2] f32.

Sharding: pure data-parallel over the flattened (B*S) row axis across 8
NeuronCores; the 2x2 table is replicated. Each core processes 2048 rows
of 2048 f32 elements (16 MB per tensor per core).

Device work per core:
  - per-row blend weights a = W[idx,0], b = W[idx,1] computed exactly on
    DVE via a = (1-idx)*W00 + idx*W10 (idx in {0,1} so each product is
    exact), using per-partition scalar operands.
  - per 128-row chunk: ACT does y *= b (activation Copy with per-partition
    scale), DVE does x = (x * a) + y in one fused scalar_tensor_tensor.
  - HWDGE (nc.sync) moves 4 MB tiles HBM<->SBUF.
"""

import numpy as np

import concourse.bacc as bacc
import concourse.bass as bass
import concourse.mybir as mybir
from concourse.bass_utils import run_bass_kernel_spmd
from concourse.tile import TileContext

B, S, D = 4, 4096, 2048
N_CORES = 8
ROWS = B * S                      # 16384
ROWS_PER_CORE = ROWS // N_CORES   # 2048
P = 128                           # SBUF partitions
GROUPS = ROWS_PER_CORE // P       # 16 row-groups of 128 rows per core
CH = 2                            # row-groups per DMA tile (2 MB tiles)
N_TILES = GROUPS // CH

F32 = mybir.dt.float32
MULT = mybir.AluOpType.mult
ADD = mybir.AluOpType.add


def _build_bass() -> bass.Bass:
    nc = bacc.Bacc(trn_type="TRN2", debug=False, enable_partition_id=False)

    x = nc.dram_tensor("x", [ROWS_PER_CORE, D], F32, kind="ExternalInput").ap()
    y = nc.dram_tensor("y", [ROWS_PER_CORE, D], F32, kind="ExternalInput").ap()
    idx = nc.dram_tensor("idx", [P, GROUPS], F32, kind="ExternalInput").ap()
    w = nc.dram_tensor("w", [P, 4], F32, kind="ExternalInput").ap()
    out = nc.dram_tensor("out", [ROWS_PER_CORE, D], F32, kind="ExternalOutput").ap()

    # Tile t covers rows [t*CH*P, (t+1)*CH*P): partition p of chunk c holds
    # row (t*CH + c)*P + p. Row-group index g = t*CH + c matches idx[:, g].
    xv = x.rearrange("(t c p) d -> t p c d", c=CH, p=P)
    yv = y.rearrange("(t c p) d -> t p c d", c=CH, p=P)
    ov = out.rearrange("(g p) d -> g p d", p=P)  # per-chunk (1 MB) stores

    with TileContext(nc) as tc:
        with (
            tc.tile_pool(name="small", bufs=1) as small,
            tc.tile_pool(name="xp", bufs=5) as xp,
            tc.tile_pool(name="yp", bufs=5) as yp,
        ):
            idx_t = small.tile([P, GROUPS], F32)
            w_t = small.tile([P, 4], F32)
            nc.sync.dma_start(out=idx_t[:], in_=idx)
            nc.sync.dma_start(out=w_t[:], in_=w)

            # nidx = 1 - idx (exact for idx in {0,1})
            nidx_t = small.tile([P, GROUPS], F32)
            nc.vector.tensor_scalar(nidx_t[:], idx_t[:], -1.0, 1.0, MULT, ADD)

            # a = nidx*W00 + idx*W10 ; b = nidx*W01 + idx*W11   (all exact)
            ta = small.tile([P, GROUPS], F32)
            tb = small.tile([P, GROUPS], F32)
            a_t = small.tile([P, GROUPS], F32)
            b_t = small.tile([P, GROUPS], F32)
            nc.vector.tensor_scalar(ta[:], idx_t[:], w_t[:, 2:3], None, MULT)
            nc.vector.scalar_tensor_tensor(a_t[:], nidx_t[:], w_t[:, 0:1], ta[:], MULT, ADD)
            nc.vector.tensor_scalar(tb[:], idx_t[:], w_t[:, 3:4], None, MULT)
            nc.vector.scalar_tensor_tensor(b_t[:], nidx_t[:], w_t[:, 1:2], tb[:], MULT, ADD)

            for t in range(N_TILES):
                xt = xp.tile([P, CH * D], F32)
                yt = yp.tile([P, CH * D], F32)
                xt3 = xt[:].rearrange("p (c d) -> p c d", c=CH)
                yt3 = yt[:].rearrange("p (c d) -> p c d", c=CH)
                # x loads on the SP HWDGE ring, y loads on the ACT HWDGE
                # ring, stores on the SWDGE (gpsimd) queue: three DMA
                # streams that overlap instead of serializing in one FIFO.
                nc.sync.dma_start(out=xt3, in_=xv[t])
                nc.scalar.dma_start(out=yt3, in_=yv[t])
                for c in range(CH):
                    g = t * CH + c
                    xs = xt[:, c * D : (c + 1) * D]
                    ys = yt[:, c * D : (c + 1) * D]
                    # Both compute passes on DVE so the DMA-dispatching
                    # engines (Sync=x loads, Scalar=y loads, GpSimd=stores)
                    # never stall on data — a stalled compute op in a
                    # dispatcher's stream head-of-line-blocks its queue.
                    nc.vector.tensor_scalar(ys, ys, b_t[:, g : g + 1], None, MULT)
                    nc.vector.scalar_tensor_tensor(
                        xs, xs, a_t[:, g : g + 1], ys, MULT, ADD
                    )
                    # store this chunk immediately (SWDGE queue)
                    nc.gpsimd.dma_start(out=ov[g], in_=xs)

    nc.compile()
    return nc


def _shard_inputs(X, Y, reward, W):
    Xf = np.ascontiguousarray(np.asarray(X, dtype=np.float32).reshape(ROWS, D))
    Yf = np.ascontiguousarray(np.asarray(Y, dtype=np.float32).reshape(ROWS, D))
    idx_all = np.asarray(reward).reshape(ROWS).astype(np.float32)
    w_rep = np.ascontiguousarray(
        np.tile(np.asarray(W, dtype=np.float32).reshape(1, 4), (P, 1))
    )
    in_maps = []
    for k in range(N_CORES):
        sl = slice(k * ROWS_PER_CORE, (k + 1) * ROWS_PER_CORE)
        # idx_core[p, g] = idx of row g*P + p of this core's shard
        idx_core = np.ascontiguousarray(idx_all[sl].reshape(GROUPS, P).T)
        in_maps.append(
            {
                "x": np.ascontiguousarray(Xf[sl]),
                "y": np.ascontiguousarray(Yf[sl]),
                "idx": idx_core,
                "w": w_rep,
            }
        )
    return in_maps


def run(X, Y, reward, W, trace=False, tmpdir=None):
    """Build, run on 8 cores; returns (full_output, BassKernelResults)."""
    in_maps = _shard_inputs(X, Y, reward, W)
    nc = _build_bass()
    res = run_bass_kernel_spmd(
        nc, in_maps, core_ids=list(range(N_CORES)), trace=trace, tmpdir=tmpdir
    )
    shards = [res.results[k]["out"] for k in range(N_CORES)]
    full = np.concatenate(shards, axis=0).reshape(B, S, D)
    return full, res


def kernel(X, Y, reward, W):
    full, _ = run(X, Y, reward, W)
    return full


# revision 14
# speedup vs baseline: 1.1653x; 1.0101x over previous
"""Trainium2 Bass kernel for nn_MultLayerAdaptiveSimple.

Computes out = X * W[idx, 0] + Y * W[idx, 1] where idx = reward[..., 0]
(values in {0, 1}), X/Y: [4, 4096, 2048] f32, W: [2, 2] f32.

Sharding: pure data-parallel over the flattened (B*S) row axis across 8
NeuronCores; the 2x2 table is replicated. Each core processes 2048 rows
of 2048 f32 elements (16 MB per tensor per core).

Device work per core:
  - per-row blend weights a = W[idx,0], b = W[idx,1] computed exactly on
    DVE via a = (1-idx)*W00 + idx*W10 (idx in {0,1} so each product is
    exact), using per-partition scalar operands.
  - per 128-row chunk: ACT does y *= b (activation Copy with per-partition
    scale), DVE does x = (x * a) + y in one fused scalar_tensor_tensor.
  - HWDGE (nc.sync) moves 4 MB tiles HBM<->SBUF.
"""

import numpy as np

import concourse.bacc as bacc
import concourse.bass as bass
import concourse.mybir as mybir
from concourse.bass_utils import run_bass_kernel_spmd
from concourse.tile import TileContext

B, S, D = 4, 4096, 2048
N_CORES = 8
ROWS = B * S                      # 16384
ROWS_PER_CORE = ROWS // N_CORES   # 2048
P = 128                           # SBUF partitions
GROUPS = ROWS_PER_CORE // P       # 16 row-groups of 128 rows per core
CH = 2                            # row-groups per DMA tile (2 MB tiles)
N_TILES = GROUPS // CH

F32 = mybir.dt.float32
MULT = mybir.AluOpType.mult
ADD = mybir.AluOpType.add


def _build_bass() -> bass.Bass:
    nc = bacc.Bacc(trn_type="TRN2", debug=False, enable_partition_id=False)

    x = nc.dram_tensor("x", [ROWS_PER_CORE, D], F32, kind="ExternalInput").ap()
    y = nc.dram_tensor("y", [ROWS_PER_CORE, D], F32, kind="ExternalInput").ap()
    idx = nc.dram_tensor("idx", [P, GROUPS], F32, kind="ExternalInput").ap()
    w = nc.dram_tensor("w", [P, 4], F32, kind="ExternalInput").ap()
    out = nc.dram_tensor("out", [ROWS_PER_CORE, D], F32, kind="ExternalOutput").ap()

    # Tile t covers rows [t*CH*P, (t+1)*CH*P): partition p of chunk c holds
    # row (t*CH + c)*P + p. Row-group index g = t*CH + c matches idx[:, g].
    xv = x.rearrange("(t c p) d -> t p c d", c=CH, p=P)
    yv = y.rearrange("(t c p) d -> t p c d", c=CH, p=P)
    ov = out.rearrange("(g p) d -> g p d", p=P)  # per-chunk (1 MB) stores

    with TileContext(nc) as tc:
        with (
            tc.tile_pool(name="small", bufs=1) as small,
            tc.tile_pool(name="xp", bufs=5) as xp,
            tc.tile_pool(name="yp", bufs=5) as yp,
        ):
            idx_t = small.tile([P, GROUPS], F32)
            w_t = small.tile([P, 4], F32)
            # On the SWDGE queue (idle until stores begin): tiny strided
            # transfers at the head of a HWDGE load ring would FIFO-delay
            # the first 2MB data loads by ~10us.
            nc.gpsimd.dma_start(out=idx_t[:], in_=idx)
            nc.gpsimd.dma_start(out=w_t[:], in_=w)

            # nidx = 1 - idx (exact for idx in {0,1})
            nidx_t = small.tile([P, GROUPS], F32)
            nc.vector.tensor_scalar(nidx_t[:], idx_t[:], -1.0, 1.0, MULT, ADD)

            # a = nidx*W00 + idx*W10 ; b = nidx*W01 + idx*W11   (all exact)
            ta = small.tile([P, GROUPS], F32)
            tb = small.tile([P, GROUPS], F32)
            a_t = small.tile([P, GROUPS], F32)
            b_t = small.tile([P, GROUPS], F32)
            nc.vector.tensor_scalar(ta[:], idx_t[:], w_t[:, 2:3], None, MULT)
            nc.vector.scalar_tensor_tensor(a_t[:], nidx_t[:], w_t[:, 0:1], ta[:], MULT, ADD)
            nc.vector.tensor_scalar(tb[:], idx_t[:], w_t[:, 3:4], None, MULT)
            nc.vector.scalar_tensor_tensor(b_t[:], nidx_t[:], w_t[:, 1:2], tb[:], MULT, ADD)

            for t in range(N_TILES):
                xt = xp.tile([P, CH * D], F32)
                yt = yp.tile([P, CH * D], F32)
                xt3 = xt[:].rearrange("p (c d) -> p c d", c=CH)
                yt3 = yt[:].rearrange("p (c d) -> p c d", c=CH)
                # x loads on the SP HWDGE ring, y loads on the ACT HWDGE
                # ring, stores on the SWDGE (gpsimd) queue: three DMA
                # streams that overlap instead of serializing in one FIFO.
                nc.sync.dma_start(out=xt3, in_=xv[t])
                nc.scalar.dma_start(out=yt3, in_=yv[t])
                for c in range(CH):
                    g = t * CH + c
                    xs = xt[:, c * D : (c + 1) * D]
                    ys = yt[:, c * D : (c + 1) * D]
                    # Both compute passes on DVE so the DMA-dispatching
                    # engines (Sync=x loads, Scalar=y loads, GpSimd=stores)
                    # never stall on data — a stalled compute op in a
                    # dispatcher's stream head-of-line-blocks its queue.
                    nc.vector.tensor_scalar(ys, ys, b_t[:, g : g + 1], None, MULT)
                    nc.vector.scalar_tensor_tensor(
                        xs, xs, a_t[:, g : g + 1], ys, MULT, ADD
                    )
                    # store this chunk immediately (SWDGE queue)
                    nc.gpsimd.dma_start(out=ov[g], in_=xs)

    nc.compile()
    return nc


def _shard_inputs(X, Y, reward, W):
    Xf = np.ascontiguousarray(np.asarray(X, dtype=np.float32).reshape(ROWS, D))
    Yf = np.ascontiguousarray(np.asarray(Y, dtype=np.float32).reshape(ROWS, D))
    idx_all = np.asarray(reward).reshape(ROWS).astype(np.float32)
    w_rep = np.ascontiguousarray(
        np.tile(np.asarray(W, dtype=np.float32).reshape(1, 4), (P, 1))
    )
    in_maps = []
    for k in range(N_CORES):
        sl = slice(k * ROWS_PER_CORE, (k + 1) * ROWS_PER_CORE)
        # idx_core[p, g] = idx of row g*P + p of this core's shard
        idx_core = np.ascontiguousarray(idx_all[sl].reshape(GROUPS, P).T)
        in_maps.append(
            {
                "x": np.ascontiguousarray(Xf[sl]),
                "y": np.ascontiguousarray(Yf[sl]),
                "idx": idx_core,
                "w": w_rep,
            }
        )
    return in_maps


def run(X, Y, reward, W, trace=False, tmpdir=None):
    """Build, run on 8 cores; returns (full_output, BassKernelResults)."""
    in_maps = _shard_inputs(X, Y, reward, W)
    nc = _build_bass()
    res = run_bass_kernel_spmd(
        nc, in_maps, core_ids=list(range(N_CORES)), trace=trace, tmpdir=tmpdir
    )
    shards = [res.results[k]["out"] for k in range(N_CORES)]
    full = np.concatenate(shards, axis=0).reshape(B, S, D)
    return full, res


def kernel(X, Y, reward, W):
    full, _ = run(X, Y, reward, W)
    return full
